# revision 1
# baseline (speedup 1.0000x reference)
"""Trainium2 Bass kernel for batched dense attention.

Reference computation (per batch b):
    q = query @ Wq + bq ; k = key @ Wk + bk ; v = value @ Wv + bv
    out = softmax(BETA * q k^T) v

Shapes: query/key/value [4, 2048, 1024], weights [1024, 1024], out [4, 2048, 1024].

Sharding: 8 cores = (batch b, seq half h). Each core computes out rows
[b, h*1024:(h+1)*1024, :] from its query shard [1024, 1024] plus the full
key/value of its batch (K/V projection duplicated across the 2 cores of a
batch; no collectives).

Core algorithm (all matmuls in float32r: ~1.5e-4 rel err, 4x fp32 speed):
  - queryT/keyT: PE-transpose raw inputs (fp32), round to f32r on the
    PSUM->SBUF copy.
  - qTr = (query @ Wq + bq)^T as [kd, q] ; kTr = (key @ Wk + bk)^T as [kd, k]
    via lhsT=W chunks (natural layout), rhs=transposed inputs; per-partition
    bias folded into the PSUM->SBUF copy.
  - S^T tiles [k, q] = lhsT(kTr).T @ rhs(qTr); exp(BETA*S^T) on ScalarE
    directly PSUM->SBUF as f32r (unnormalized probabilities pT).
  - row sums via PE: lhsT=pT slice, rhs=ones -> [q, 1] accumulated in PSUM.
  - out2 = pT.T @ value (value streamed from DRAM, contraction over k).
  - out = (out2 @ Wv) * (1/rowsum) + bv  -- normalization deferred to the
    end (linear), applied with a fused scalar_tensor_tensor on VectorE.
  - out2^T via PE transposes of o2 blocks (f32r, packs of 4 per PSUM bank).
"""
import ml_dtypes
import numpy as np

import concourse.bass as bass
import concourse.bacc as bacc
import concourse.tile as tile
from concourse import masks, mybir
from concourse.tile import add_dep_helper
from concourse.bass_utils import run_bass_kernel_spmd

B, S, D = 4, 2048, 1024
KD = 1024  # key_dim == value_dim == D
VD = 1024
BETA = 1.0 / float(np.sqrt(D))
N_CORES = 8
QS = S // 2  # per-core query rows (1024)

F32 = mybir.dt.float32
F32R = mybir.dt.float32r
BF16 = mybir.dt.bfloat16

C_D = D // 128     # 8 contraction chunks over D
G_KD = KD // 128   # 8 kd chunks
KT = S // 128      # 16 key tiles
QBLK = 512         # q-block size
NQB = QS // QBLK   # 4 q blocks
NQS = QBLK // 128  # 2 q slices per block


DEBUG_TAPS = False


def build_kernel():
    nc = bacc.Bacc("TRN2", target_bir_lowering=False, debug=False,
                   num_devices=N_CORES)

    q_sh = nc.dram_tensor("q_sh", [QS, D], F32, kind="ExternalInput").ap()
    key_b = nc.dram_tensor("key_b", [S, D], F32, kind="ExternalInput").ap()
    val_b = nc.dram_tensor("val_b", [S, D], F32, kind="ExternalInput").ap()
    Wq = nc.dram_tensor("Wq", [D, KD], F32, kind="ExternalInput").ap()
    Wk = nc.dram_tensor("Wk", [D, KD], F32, kind="ExternalInput").ap()
    Wv = nc.dram_tensor("Wv", [D, VD], F32, kind="ExternalInput").ap()
    bq = nc.dram_tensor("bq", [KD], F32, kind="ExternalInput").ap()
    bk = nc.dram_tensor("bk", [KD], F32, kind="ExternalInput").ap()
    bv = nc.dram_tensor("bv", [VD], F32, kind="ExternalInput").ap()
    out = nc.dram_tensor("out", [QS, VD], F32, kind="ExternalOutput").ap()
    taps = None
    if DEBUG_TAPS:
        taps = {
            "t_qTr": nc.dram_tensor("t_qTr", [128, G_KD * QS], F32,
                                    kind="ExternalOutput").ap(),
            "t_kTr": nc.dram_tensor("t_kTr", [128, G_KD * S], F32,
                                    kind="ExternalOutput").ap(),
            "t_pT": nc.dram_tensor("t_pT", [128, KT * QBLK], BF16,
                                   kind="ExternalOutput").ap(),
            "t_rs": nc.dram_tensor("t_rs", [128, 2 * NQS], F32,
                                   kind="ExternalOutput").ap(),
            "t_o2T": nc.dram_tensor("t_o2T", [128, C_D * QBLK], F32,
                                    kind="ExternalOutput").ap(),
        }

    with tile.TileContext(nc) as tc:
        _body(tc, q_sh, key_b, val_b, Wq, Wk, Wv, bq, bk, bv, out, taps)
    nc.compile()
    return nc


def _body(tc, q_sh, key_b, val_b, Wq, Wk, Wv, bq, bk, bv, out, taps=None):
    nc = tc.nc
    Exp = mybir.ActivationFunctionType.Exp
    mult = mybir.AluOpType.mult
    add = mybir.AluOpType.add

    # ---- consolidated persistent constants (two tiles: f32 / f32r) ------
    # constf cols: [0:8]=bqT, [8:16]=bkT, [16:16+VD]=bvb,
    #              [1040] ones col, row0 [1048:1048+VD]=bv staging
    const_pool = tc.alloc_tile_pool(name="const", bufs=1)
    constf = const_pool.tile([128, 1184], F32, name="constf")
    bqT = constf[:, 0:8]
    bkT = constf[:, 8:16]
    bvb = constf[:, 16:16 + VD]
    ones_f = constf[:, 1040:1042]
    onesrow_f = constf[0:1, 1041:1041 + 128]
    bv_f = constf[0:1, 16:16 + VD]
    rrec_all = constf[:, 1168:1168 + 2 * (QS // 128)]  # per-qs recip columns
    # constr cols: [0:128]=ident_r, row0 [136:136+VD]=bv_r,
    #              [1164:1164+128] onesrow_r
    constr = const_pool.tile([128, 1312], F32R, name="constr")
    ident_r = constr[:, 0:128]
    bv_r = constr[0:1, 136:136 + VD]
    onesrow_r = constr[0:1, 1164:1164 + 128]
    onesb = const_pool.tile([128, 2], BF16, name="onesb")

    for c in range(G_KD):
        nc.sync.dma_start(out=bqT[:, c:c + 1], in_=bq[c * 128:(c + 1) * 128])
        nc.sync.dma_start(out=bkT[:, c:c + 1], in_=bk[c * 128:(c + 1) * 128])
    nc.sync.dma_start(out=bv_f, in_=bv[:])
    nc.vector.memset(ones_f, 1.0)
    nc.vector.memset(onesrow_f, 1.0)
    nc.vector.tensor_copy(onesb[:], ones_f)
    nc.vector.tensor_copy(onesrow_r, onesrow_f)
    nc.vector.tensor_copy(bv_r, bv_f)

    # persistent big activations (allocated early: released late, LIFO)
    big_pool = tc.alloc_tile_pool(name="big", bufs=1)
    qTr = big_pool.tile([128, G_KD * QS], F32R, name="qTr")      # 32KB/p
    kTr = big_pool.tile([128, G_KD * S], F32R, name="kTr")       # 64KB/p

    # prologue-only constants (fp32 identity for raw transposes)
    pro_pool = tc.alloc_tile_pool(name="pro", bufs=1)
    ident_f = pro_pool.tile([128, 128], F32, name="ident_f")
    masks.make_identity(nc, ident_f[:])
    nc.vector.tensor_copy(ident_r, ident_f[:])

    psA = tc.alloc_tile_pool(name="psA", bufs=1, space="PSUM")

    # bv broadcast to all partitions via K=1 matmul
    for n in range(VD // 512):
        bc_ps = psA.tile([128, 512], F32, name="bc_ps", tag="mm", bufs=2)
        nc.tensor.matmul(bc_ps[:], onesrow_r,
                         bv_r[:, n * 512:(n + 1) * 512],
                         start=True, stop=True)
        nc.vector.tensor_copy(bvb[:, n * 512:(n + 1) * 512], bc_ps[:])

    # ===== P-K: key transpose + k projection (PE transposes) ==============
    HALF = S // 2
    wk_pool = tc.alloc_tile_pool(name="wk", bufs=1)
    Wkr = wk_pool.tile([128, C_D * KD], F32R, name="Wkr")
    for c in range(C_D):
        nc.gpsimd.dma_start(out=Wkr[:, c * KD:(c + 1) * KD],
                            in_=Wk[c * 128:(c + 1) * 128, :])

    kt_pool = tc.alloc_tile_pool(name="kt", bufs=1)
    n_krow = HALF // 128
    for kh in range(2):
        keyT = kt_pool.tile([128, C_D * HALF], F32R, name="keyT",
                            tag="keyT", bufs=1)
        for rt in range(n_krow):
            krow = kt_pool.tile([128, D], F32, name="krow", tag="krow", bufs=4)
            nc.sync.dma_start(
                out=krow[:],
                in_=key_b[kh * HALF + rt * 128:kh * HALF + (rt + 1) * 128, :])
            for cg in range(2):
                ktp_ps = psA.tile([128, 512], F32, name="ktp_ps", tag="tp",
                                  bufs=4)
                for j in range(4):
                    c = cg * 4 + j
                    nc.tensor.transpose(ktp_ps[:, j * 128:(j + 1) * 128],
                                        krow[:, c * 128:(c + 1) * 128],
                                        ident_f[:])
                nc.vector.tensor_copy(
                    keyT[:, rt * D + cg * 512:rt * D + (cg + 1) * 512],
                    ktp_ps[:])
        kT_v = keyT[:].rearrange("p (rt x) -> p rt x", rt=n_krow)
        for g in range(G_KD):
            for nt in range(HALF // 512):
                kmm_ps = psA.tile([128, 512], F32, name="kmm_ps", tag="mm",
                                  bufs=2)
                for c in range(C_D):
                    nc.tensor.matmul(
                        kmm_ps[:],
                        Wkr[:, c * KD + g * 128:c * KD + (g + 1) * 128],
                        kT_v[:, nt * 4:(nt + 1) * 4, c * 128:(c + 1) * 128],
                        start=(c == 0), stop=(c == C_D - 1))
                nc.vector.tensor_scalar(
                    out=kTr[:, g * S + kh * HALF + nt * 512:
                            g * S + kh * HALF + (nt + 1) * 512],
                    in0=kmm_ps[:], scalar1=bkT[:, g:g + 1], scalar2=None,
                    op0=add)
    kt_pool.release()
    wk_pool.release()

    # ===== P-Q: Wq load, query transpose, q projection ====================
    wq_pool = tc.alloc_tile_pool(name="wq", bufs=1)
    Wqr = wq_pool.tile([128, C_D * KD], F32R, name="Wqr")
    for c in range(C_D):
        nc.gpsimd.dma_start(out=Wqr[:, c * KD:(c + 1) * KD],
                            in_=Wq[c * 128:(c + 1) * 128, :])

    qt_pool = tc.alloc_tile_pool(name="qt", bufs=1)
    queryT = qt_pool.tile([128, C_D * QS], F32R, name="queryT")
    n_qrow = QS // 128
    for rt in range(n_qrow):
        qrow = qt_pool.tile([128, D], F32, name="qrow", tag="qrow", bufs=4)
        nc.sync.dma_start(out=qrow[:], in_=q_sh[rt * 128:(rt + 1) * 128, :])
        for cg in range(2):
            tp_ps = psA.tile([128, 512], F32, name="tp_ps", tag="tp", bufs=4)
            for j in range(4):
                c = cg * 4 + j
                nc.tensor.transpose(tp_ps[:, j * 128:(j + 1) * 128],
                                    qrow[:, c * 128:(c + 1) * 128], ident_f[:])
            nc.vector.tensor_copy(
                queryT[:, rt * D + cg * 512:rt * D + (cg + 1) * 512], tp_ps[:])

    qT_v = queryT[:].rearrange("p (rt x) -> p rt x", rt=n_qrow)
    for g in range(G_KD):
        for nt in range(QS // 512):
            mm_ps = psA.tile([128, 512], F32, name="mm_ps", tag="mm", bufs=2)
            for c in range(C_D):
                nc.tensor.matmul(
                    mm_ps[:],
                    Wqr[:, c * KD + g * 128:c * KD + (g + 1) * 128],
                    qT_v[:, nt * 4:(nt + 1) * 4, c * 128:(c + 1) * 128],
                    start=(c == 0), stop=(c == C_D - 1))
            nc.vector.tensor_scalar(
                out=qTr[:, g * QS + nt * 512:g * QS + (nt + 1) * 512],
                in0=mm_ps[:], scalar1=bqT[:, g:g + 1], scalar2=None, op0=add)
    qt_pool.release()
    wq_pool.release()
    psA.release()
    pro_pool.release()

    # ===== P6: attention main loop ========================================
    # All PSUM and SBUF working tiles are created ONCE and reused via
    # same-tile WAR dependencies (manual rotation). Dynamic pool-slot
    # handoff between independent chains can deadlock the Tile scheduler
    # (in-order engines + slot-wait cycles), so P6 avoids it entirely.
    # PSUM: sT(2) + rs(1) + o2(4) + op(1) = 8 banks.
    psB = tc.alloc_tile_pool(name="psB", bufs=1, space="PSUM")
    sT_tiles = [psB.tile([128, QBLK], F32, name=f"sT{i}", tag=f"sT{i}")
                for i in range(2)]
    rs_ps = psB.tile([128, 2 * NQS], F32, name="rs_ps", tag="rs")
    o2_tiles = [psB.tile([128, 512], F32, name=f"o2_{i}", tag=f"o2_{i}")
                for i in range(NQS)]
    op_ps = psB.tile([128, 512], F32R, name="op_ps", tag="opb")
    op_f32 = op_ps[:].bitcast(F32)

    mn_pool = tc.alloc_tile_pool(name="mn", bufs=1)
    pT_tiles = [mn_pool.tile([128, KT * QBLK], BF16, name=f"pT{i}",
                             tag=f"pT{i}") for i in range(2)]
    o2T = mn_pool.tile([128, C_D * QBLK], F32R, name="o2T")
    o2r_all = mn_pool.tile([128, NQS * 512], F32R, name="o2r_all")
    o2r_tiles = [o2r_all[:, i * 512:(i + 1) * 512] for i in range(NQS)]
    vch = mn_pool.tile([128, 8 * 512], BF16, name="vch")  # 8-slice value ring
    ost_all = mn_pool.tile([128, 2 * 512], F32, name="ost_all")
    ostage_tiles = [ost_all[:, i * 512:(i + 1) * 512] for i in range(2)]
    Wvh_tiles = [mn_pool.tile([128, C_D * 512], F32R, name=f"Wvh{i}",
                              tag=f"Wvh{i}") for i in range(2)]
    for vd in range(2):
        for c in range(C_D):
            nc.gpsimd.dma_start(
                out=Wvh_tiles[vd][:, c * 512:(c + 1) * 512],
                in_=Wv[c * 128:(c + 1) * 128, vd * 512:(vd + 1) * 512])

    for qb in range(NQB):
        q0 = qb * QBLK
        pT = pT_tiles[qb % 2]
        # ---- phase A: S^T -> exp -> pT ; rowsums ----
        for kt in range(KT):
            sT_ps = sT_tiles[kt % 2]
            for g in range(G_KD):
                nc.tensor.matmul(
                    sT_ps[:],
                    kTr[:, g * S + kt * 128:g * S + (kt + 1) * 128],
                    qTr[:, g * QS + q0:g * QS + q0 + QBLK],
                    start=(g == 0), stop=(g == G_KD - 1))
            nc.scalar.activation(pT[:, kt * QBLK:(kt + 1) * QBLK], sT_ps[:],
                                 Exp, scale=float(BETA))
            for qs in range(NQS):
                # single whole-bank clear on the very first rs matmul; the
                # other groups' first writes land on cleared has_written bits
                nc.tensor.matmul(
                    rs_ps[:, 2 * qs:2 * qs + 2],
                    pT[:, kt * QBLK + qs * 128:kt * QBLK + (qs + 1) * 128],
                    onesb[:],
                    start=(kt == 0 and qs == 0),
                    stop=(kt == KT - 1 and qs == NQS - 1),
                    skip_group_check=True)
        rrec = rrec_all[:, qb * 2 * NQS:(qb + 1) * 2 * NQS]
        if taps is not None and qb == 0:
            nc.sync.dma_start(out=taps["t_pT"][:], in_=pT[:])
            trs = mn_pool.tile([128, 2 * NQS], F32, name="trs")
            nc.vector.tensor_copy(trs[:], rs_ps[:])
            nc.sync.dma_start(out=taps["t_rs"][:], in_=trs[:])
        nc.vector.reciprocal(rrec, rs_ps[:])

        # ---- phase B: out2 = pT.T @ value (vd-outer, value streamed) ----
        for vd in range(2):
            for kt in range(KT):
                nc.gpsimd.dma_start(
                    out=vch[:, (kt % 8) * 512:(kt % 8 + 1) * 512],
                    in_=val_b[kt * 128:(kt + 1) * 128,
                              vd * 512:(vd + 1) * 512])
                for qs in range(NQS):
                    nc.tensor.matmul(
                        o2_tiles[qs][:],
                        pT[:, kt * QBLK + qs * 128:kt * QBLK + (qs + 1) * 128],
                        vch[:, (kt % 8) * 512:(kt % 8 + 1) * 512],
                        start=(kt == 0), stop=(kt == KT - 1))
            # free the o2 PSUM tiles first (copies with no PE deps), THEN
            # transpose+scatter
            for qs in range(NQS):
                nc.vector.tensor_copy(o2r_tiles[qs][:], o2_tiles[qs][:])
            for qs in range(NQS):
                o2r = o2r_tiles[qs]
                for u in range(4):
                    nc.tensor.transpose(op_ps[:, u * 128:(u + 1) * 128],
                                        o2r[:, u * 128:(u + 1) * 128],
                                        ident_r)
                # o2T[:, (vd*4+u)*QBLK + qs*128 : +128] <- op_ps[:, u*128:+128]
                src_ap = op_ps[:].rearrange("p (u f) -> p u f", u=4)
                dst = o2T[:].rearrange("p (c f) -> p c f", c=C_D)[
                    :, vd * 4:(vd + 1) * 4, qs * 128:(qs + 1) * 128]
                nc.vector.tensor_copy(dst, src_ap)

        if taps is not None and qb == 0:
            nc.sync.dma_start(out=taps["t_o2T"][:], in_=o2T[:].bitcast(F32))

        # ---- phase C: out = (out2 @ Wv) * rrec + bv ----
        for vd in range(2):
            Wvh = Wvh_tiles[vd]
            for qs in range(NQS):
                for c in range(C_D):
                    nc.tensor.matmul(
                        op_f32,
                        o2T[:, c * QBLK + qs * 128:c * QBLK + (qs + 1) * 128],
                        Wvh[:, c * 512:(c + 1) * 512],
                        start=(c == 0), stop=(c == C_D - 1))
                ostage = ostage_tiles[qs % 2]
                nc.vector.scalar_tensor_tensor(
                    out=ostage[:], in0=op_f32, scalar=rrec[:, 2 * qs:2 * qs + 1],
                    in1=bvb[:, vd * 512:(vd + 1) * 512], op0=mult, op1=add)
                nc.sync.dma_start(
                    out=out[q0 + qs * 128:q0 + (qs + 1) * 128,
                            vd * 512:(vd + 1) * 512],
                    in_=ostage[:])

    mn_pool.release()
    psB.release()
    big_pool.release()
    const_pool.release()


_NC_CACHE = {}


def _get_nc():
    if "nc" not in _NC_CACHE:
        _NC_CACHE["nc"] = build_kernel()
    return _NC_CACHE["nc"]


def kernel(query, key, value, Wq, bq, Wk, bk, Wv, bv):
    query = np.ascontiguousarray(np.asarray(query, dtype=np.float32))
    key = np.ascontiguousarray(np.asarray(key, dtype=np.float32))
    value = np.ascontiguousarray(np.asarray(value, dtype=np.float32))
    Wq = np.ascontiguousarray(np.asarray(Wq, dtype=np.float32))
    Wk = np.ascontiguousarray(np.asarray(Wk, dtype=np.float32))
    Wv = np.ascontiguousarray(np.asarray(Wv, dtype=np.float32))
    bq = np.ascontiguousarray(np.asarray(bq, dtype=np.float32))
    bk = np.ascontiguousarray(np.asarray(bk, dtype=np.float32))
    bv = np.ascontiguousarray(np.asarray(bv, dtype=np.float32))

    nc = _get_nc()
    in_maps = make_in_maps(query, key, value, Wq, bq, Wk, bk, Wv, bv)
    res = run_bass_kernel_spmd(nc, in_maps, list(range(N_CORES)))
    outp = np.empty((B, S, VD), dtype=np.float32)
    for core in range(N_CORES):
        b, h = divmod(core, 2)
        outp[b, h * QS:(h + 1) * QS, :] = res.results[core]["out"]
    return outp


def make_in_maps(query, key, value, Wq, bq, Wk, bk, Wv, bv):
    in_maps = []
    for core in range(N_CORES):
        b, h = divmod(core, 2)
        in_maps.append({
            "q_sh": np.ascontiguousarray(query[b, h * QS:(h + 1) * QS, :]),
            "key_b": key[b],
            "val_b": value[b],
            "Wq": Wq, "Wk": Wk, "Wv": Wv,
            "bq": bq, "bk": bk, "bv": bv,
        })
    return in_maps



# revision 3
# speedup vs baseline: 1.2418x; 1.2418x over previous
"""Trainium2 Bass kernel for batched dense attention (v2).

Reference (per batch b):
    q = query @ Wq + bq ; k = key @ Wk + bk ; v = value @ Wv + bv
    out = softmax(BETA * q k^T) v

Sharding: 8 cores = (batch b, seq half h). Core (b,h) computes out rows
[b, h*1024:(h+1)*1024, :]. K-side work is duplicated across the two cores
of a batch (no collectives).

v2 changes vs v1 (403us -> target ~250us):
  - query/key arrive HOST-TRANSPOSED ([D, rows] layout) so the kernel needs
    ZERO PE transposes (v1 spent ~40us of PE time on 256 transposes).
  - value/Wv arrive host-cast to bf16; all matmul operands are bf16 except
    the projection inputs (f32 data issued as float32r) -- same 1 col/cycle
    PE rate, half the SBUF/DMA footprint.
  - Projections stream W/input chunks c-outer into 8 PSUM banks so PE work
    starts ~2us into the kernel (v1 idled 27us before the first matmul).
  - Phase B computes out2^T = (value^T P)^T directly by using natural-layout
    value tiles as lhsT (contraction over k), removing v1's PSUM transpose
    round-trip; phase C then consumes out2T as lhsT with natural Wv as rhs.
  - Softmax normalization deferred to the output copy (mult by 1/rowsum
    fused with +bv in one scalar_tensor_tensor), as v1.

Per-core PE cycle budget @2.4GHz: qproj 65k + kproj 131k + scores 131k +
phaseB 131k + phaseC 65k ~= 523k cycles ~= 218us.
"""
import ml_dtypes
import numpy as np

import concourse.bass as bass
import concourse.bacc as bacc
import concourse.tile as tile
from concourse import mybir
from concourse.bass_utils import run_bass_kernel_spmd

B, S, D = 4, 2048, 1024
KD = 1024
VD = 1024
BETA = 1.0 / float(np.sqrt(D))
N_CORES = 8
QS = S // 2          # per-core query rows (1024)

F32 = mybir.dt.float32
F32R = mybir.dt.float32r
BF16 = mybir.dt.bfloat16

C_D = D // 128       # 8 contraction chunks over D
G_KD = KD // 128     # 8 kd chunks
KT = S // 128        # 16 key tiles
QBLK = 512
NQB = QS // QBLK     # 2 q blocks
NQS = QBLK // 128    # 4 q slices per block


def build_kernel():
    nc = bacc.Bacc("TRN2", target_bir_lowering=False, debug=False,
                   num_devices=N_CORES)

    qT = nc.dram_tensor("qT", [D, QS], BF16, kind="ExternalInput").ap()
    kT = nc.dram_tensor("kT", [D, S], BF16, kind="ExternalInput").ap()
    v16 = nc.dram_tensor("v16", [S, VD], BF16, kind="ExternalInput").ap()
    Wq = nc.dram_tensor("Wq", [D, KD], BF16, kind="ExternalInput").ap()
    Wk = nc.dram_tensor("Wk", [D, KD], BF16, kind="ExternalInput").ap()
    Wv16 = nc.dram_tensor("Wv16", [VD, VD], BF16, kind="ExternalInput").ap()
    bq = nc.dram_tensor("bq", [KD], F32, kind="ExternalInput").ap()
    bk = nc.dram_tensor("bk", [KD], F32, kind="ExternalInput").ap()
    bv = nc.dram_tensor("bv", [VD], F32, kind="ExternalInput").ap()
    out = nc.dram_tensor("out", [QS, VD], F32, kind="ExternalOutput").ap()

    with tile.TileContext(nc) as tc:
        _body(tc, qT, kT, v16, Wq, Wk, Wv16, bq, bk, bv, out)
    nc.compile()
    return nc


def _body(tc, qT, kT, v16, Wq, Wk, Wv16, bq, bk, bv, out):
    nc = tc.nc
    Exp = mybir.ActivationFunctionType.Exp
    mult = mybir.AluOpType.mult
    add = mybir.AluOpType.add

    # ---- persistent constants ------------------------------------------
    # constf cols: [0:8]=bqT, [8:16]=bkT, [16:1040]=bvb (bv broadcast),
    # [1040:1042]=ones, [1042:1058]=rrec (2 qb x 8), row0 [1058:1186]=ones
    # row, row0 [1186:2210]=bv staging
    const_pool = tc.alloc_tile_pool(name="const", bufs=1)
    constf = const_pool.tile([128, 2210], F32, name="constf")
    bqT = constf[:, 0:8]
    bkT = constf[:, 8:16]
    bvb = constf[:, 16:16 + VD]
    ones_f = constf[:, 1040:1042]
    rrec_all = constf[:, 1042:1058]
    onesrow_f = constf[0:1, 1058:1058 + 128]
    bv_stage = constf[0:1, 1186:1186 + VD]
    onesb = const_pool.tile([128, 2], BF16, name="onesb")

    # biases / ones on the scalar queue (sync queue is for Wq/qT)
    for c in range(G_KD):
        nc.scalar.dma_start(out=bqT[:, c:c + 1], in_=bq[c * 128:(c + 1) * 128])
        nc.scalar.dma_start(out=bkT[:, c:c + 1], in_=bk[c * 128:(c + 1) * 128])
    nc.scalar.dma_start(out=bv_stage, in_=bv[:])
    nc.vector.memset(ones_f, 1.0)
    nc.vector.memset(onesrow_f, 1.0)
    nc.vector.tensor_copy(onesb[:], ones_f)

    # ---- persistent activations ----------------------------------------
    big_pool = tc.alloc_tile_pool(name="big", bufs=1)
    qTr = big_pool.tile([128, G_KD * QS], BF16, name="qTr")       # 16KB/p
    kTr = big_pool.tile([128, G_KD * S], BF16, name="kTr")        # 32KB/p
    Wv_sb = big_pool.tile([128, C_D * VD], BF16, name="Wv_sb")    # 16KB/p
    pT = big_pool.tile([128, KT * QBLK], BF16, name="pT")         # 16KB/p
    o2T = big_pool.tile([128, C_D * QBLK], BF16, name="o2T")      # 8KB/p
    ost_all = big_pool.tile([128, 2 * 512], F32, name="ost_all")  # 4KB/p
    ostage = [ost_all[:, i * 512:(i + 1) * 512] for i in range(2)]

    for c in range(C_D):
        nc.gpsimd.dma_start(out=Wv_sb[:, c * VD:(c + 1) * VD],
                            in_=Wv16[c * 128:(c + 1) * 128, :])

    # ---- projection-phase transients -----------------------------------
    proj_pool = tc.alloc_tile_pool(name="proj", bufs=1)
    Wq_sb = proj_pool.tile([128, C_D * KD], BF16, name="Wq_sb")   # 16KB/p
    Wk_sb = proj_pool.tile([128, C_D * KD], BF16, name="Wk_sb")   # 16KB/p

    psPro = tc.alloc_tile_pool(name="psPro", bufs=1, space="PSUM")

    # bv broadcast to all partitions via K=1 matmul (plain fp32, prologue)
    for n in range(VD // 512):
        bc_ps = psPro.tile([128, 512], F32, name="bc_ps", tag="pp", bufs=8)
        nc.tensor.matmul(bc_ps[:], onesrow_f,
                         bv_stage[:, n * 512:(n + 1) * 512],
                         start=True, stop=True)
        nc.vector.tensor_copy(bvb[:, n * 512:(n + 1) * 512], bc_ps[:])

    # ---- q projection: qTr[kd, q] = (Wq^T qT) + bq ----------------------
    # c-outer streaming: interleave Wq chunk c with the qT block-0 chunk c
    # on the sync queue so the first matmul fires ~2us in.
    def ring_tile(engine, src_ap, name, tag, bufs):
        t = proj_pool.tile([128, 512], BF16, name=name, tag=tag, bufs=bufs)
        engine.dma_start(out=t[:], in_=src_ap)
        return t

    NKB = S // 512   # 4 key col-blocks
    kring = {}
    for c in range(C_D):
        nc.sync.dma_start(out=Wq_sb[:, c * KD:(c + 1) * KD],
                          in_=Wq[c * 128:(c + 1) * 128, :])
        nc.scalar.dma_start(out=Wk_sb[:, c * KD:(c + 1) * KD],
                            in_=Wk[c * 128:(c + 1) * 128, :])

    for blk in range(QS // 512):
        qring = [ring_tile(nc.sync,
                           qT[c * 128:(c + 1) * 128, blk * 512:(blk + 1) * 512],
                           f"qr{blk}_{c}", "qring", 4)
                 for c in range(C_D)]
        if blk == 0:  # early kT prefetch on the scalar queue
            for c in range(C_D):
                kring[(0, c)] = ring_tile(
                    nc.scalar, kT[c * 128:(c + 1) * 128, 0:512], f"kr0_{c}",
                    "kring", 6)
        pps = [psPro.tile([128, 512], F32, name=f"qp{blk}_{g}", tag="pp",
                          bufs=8) for g in range(G_KD)]
        for c in range(C_D):
            for g in range(G_KD):
                nc.tensor.matmul(
                    pps[g][:],
                    Wq_sb[:, c * KD + g * 128:c * KD + (g + 1) * 128],
                    qring[c][:],
                    start=(c == 0), stop=(c == C_D - 1))
        for g in range(G_KD):
            nc.vector.tensor_scalar(
                out=qTr[:, g * QS + blk * 512:g * QS + (blk + 1) * 512],
                in0=pps[g][:], scalar1=bqT[:, g:g + 1], scalar2=None, op0=add)

    # ---- k projection: kTr[kd, k] = (Wk^T kT) + bk ----------------------
    for blk in range(NKB):
        kring_b = []
        for c in range(C_D):
            if (blk, c) in kring:
                kring_b.append(kring[(blk, c)])
            else:
                eng = nc.scalar if blk < 2 else nc.sync
                kring_b.append(ring_tile(
                    eng, kT[c * 128:(c + 1) * 128, blk * 512:(blk + 1) * 512],
                    f"kr{blk}_{c}", "kring", 6))
        pps = [psPro.tile([128, 512], F32, name=f"kp{blk}_{g}", tag="pp",
                          bufs=8) for g in range(G_KD)]
        for c in range(C_D):
            for g in range(G_KD):
                nc.tensor.matmul(
                    pps[g][:],
                    Wk_sb[:, c * KD + g * 128:c * KD + (g + 1) * 128],
                    kring_b[c][:],
                    start=(c == 0), stop=(c == C_D - 1))
        for g in range(G_KD):
            nc.vector.tensor_scalar(
                out=kTr[:, g * S + blk * 512:g * S + (blk + 1) * 512],
                in0=pps[g][:], scalar1=bkT[:, g:g + 1], scalar2=None, op0=add)

    psPro.release()
    proj_pool.release()

    # ===== main attention loop ==========================================
    # PSUM: sT(2) + rs(1) + acc(4) = 7 banks.
    psM = tc.alloc_tile_pool(name="psM", bufs=1, space="PSUM")
    rs_ps = psM.tile([128, 2 * NQS], F32, name="rs_ps", tag="rs")

    for qb in range(NQB):
        q0 = qb * QBLK
        # ---- phase A: sT = kTr^T qTr -> exp -> pT ; rowsums on PE ------
        for kt in range(KT):
            sT = psM.tile([128, QBLK], F32, name=f"sT{qb}_{kt}", tag="sT",
                          bufs=2)
            for g in range(G_KD):
                nc.tensor.matmul(
                    sT[:],
                    kTr[:, g * S + kt * 128:g * S + (kt + 1) * 128],
                    qTr[:, g * QS + q0:g * QS + q0 + QBLK],
                    start=(g == 0), stop=(g == G_KD - 1))
            nc.scalar.activation(pT[:, kt * QBLK:(kt + 1) * QBLK], sT[:],
                                 Exp, scale=float(BETA))
            for qs in range(NQS):
                nc.tensor.matmul(
                    rs_ps[:, 2 * qs:2 * qs + 2],
                    pT[:, kt * QBLK + qs * 128:kt * QBLK + (qs + 1) * 128],
                    onesb[:],
                    start=(kt == 0 and qs == 0),
                    stop=(kt == KT - 1 and qs == NQS - 1),
                    skip_group_check=True)
        rrec = rrec_all[:, qb * 2 * NQS:(qb + 1) * 2 * NQS]
        nc.vector.reciprocal(rrec, rs_ps[:])

        # ---- phase B: o2T[vd', q] = (value^T P)^T via lhsT=value tiles --
        for p in range(2):
            accs = [psM.tile([128, QBLK], F32, name=f"o2{qb}_{p}_{u}",
                             tag="acc", bufs=4) for u in range(4)]
            for kt in range(KT):
                vt = big_pool.tile([128, 512], BF16, name=f"vt{qb}_{p}_{kt}",
                                   tag="vring", bufs=8)
                nc.gpsimd.dma_start(
                    out=vt[:],
                    in_=v16[kt * 128:(kt + 1) * 128, p * 512:(p + 1) * 512])
                for u in range(4):
                    nc.tensor.matmul(
                        accs[u][:], vt[:, u * 128:(u + 1) * 128],
                        pT[:, kt * QBLK:(kt + 1) * QBLK],
                        start=(kt == 0), stop=(kt == KT - 1))
            for u in range(4):
                nc.vector.tensor_copy(
                    o2T[:, (4 * p + u) * QBLK:(4 * p + u + 1) * QBLK],
                    accs[u][:])

        # ---- phase C: out = (o2T^T Wv) * rrec + bv ----------------------
        for qs in range(NQS):
            for col in range(2):
                op = psM.tile([128, 512], F32, name=f"op{qb}_{qs}_{col}",
                              tag="acc", bufs=4)
                for cp in range(C_D):
                    nc.tensor.matmul(
                        op[:],
                        o2T[:, cp * QBLK + qs * 128:cp * QBLK + (qs + 1) * 128],
                        Wv_sb[:, cp * VD + col * 512:cp * VD + (col + 1) * 512],
                        start=(cp == 0), stop=(cp == C_D - 1))
                ost = ostage[(2 * qs + col) % 2]
                nc.vector.scalar_tensor_tensor(
                    out=ost, in0=op[:], scalar=rrec[:, 2 * qs:2 * qs + 1],
                    in1=bvb[:, col * 512:(col + 1) * 512], op0=mult, op1=add)
                nc.sync.dma_start(
                    out=out[q0 + qs * 128:q0 + (qs + 1) * 128,
                            col * 512:(col + 1) * 512],
                    in_=ost)

    psM.release()
    big_pool.release()
    const_pool.release()


_NC_CACHE = {}


def _get_nc():
    if "nc" not in _NC_CACHE:
        _NC_CACHE["nc"] = build_kernel()
    return _NC_CACHE["nc"]


def kernel(query, key, value, Wq, bq, Wk, bk, Wv, bv):
    query = np.asarray(query, dtype=np.float32)
    key = np.asarray(key, dtype=np.float32)
    value = np.asarray(value, dtype=np.float32)
    Wq = np.ascontiguousarray(np.asarray(Wq, dtype=np.float32))
    Wk = np.ascontiguousarray(np.asarray(Wk, dtype=np.float32))
    Wv = np.asarray(Wv, dtype=np.float32)
    bq = np.ascontiguousarray(np.asarray(bq, dtype=np.float32))
    bk = np.ascontiguousarray(np.asarray(bk, dtype=np.float32))
    bv = np.ascontiguousarray(np.asarray(bv, dtype=np.float32))

    nc = _get_nc()
    in_maps = make_in_maps(query, key, value, Wq, bq, Wk, bk, Wv, bv)
    res = run_bass_kernel_spmd(nc, in_maps, list(range(N_CORES)))
    outp = np.empty((B, S, VD), dtype=np.float32)
    for core in range(N_CORES):
        b, h = divmod(core, 2)
        outp[b, h * QS:(h + 1) * QS, :] = res.results[core]["out"]
    return outp


def make_in_maps(query, key, value, Wq, bq, Wk, bk, Wv, bv):
    bf16 = ml_dtypes.bfloat16
    Wq16 = Wq.astype(bf16)
    Wk16 = Wk.astype(bf16)
    Wv16 = Wv.astype(bf16)
    kTs = [np.ascontiguousarray(key[b].T.astype(bf16)) for b in range(B)]
    v16s = [np.ascontiguousarray(value[b].astype(bf16)) for b in range(B)]
    in_maps = []
    for core in range(N_CORES):
        b, h = divmod(core, 2)
        in_maps.append({
            "qT": np.ascontiguousarray(query[b, h * QS:(h + 1) * QS, :].T
                                       .astype(bf16)),
            "kT": kTs[b],
            "v16": v16s[b],
            "Wq": Wq16, "Wk": Wk16, "Wv16": Wv16,
            "bq": bq, "bk": bk, "bv": bv,
        })
    return in_maps


# revision 4
# speedup vs baseline: 1.2479x; 1.0048x over previous
"""Trainium2 Bass kernel for batched dense attention (v3).

Reference (per batch b):
    q = query @ Wq + bq ; k = key @ Wk + bk ; v = value @ Wv + bv
    out = softmax(BETA * q k^T) v

Sharding: 8 cores = (batch b, seq half h). Core (b,h) computes out rows
[b, h*1024:(h+1)*1024, :]. K-side work is duplicated across the two cores
of a batch (no collectives).

Design (v2/v3):
  - query/key arrive HOST-TRANSPOSED ([D, rows]) and all matmul inputs are
    host-cast bf16, so the kernel needs zero PE transposes and no dtype
    juggling; PSUM accumulates in f32.
  - Projections stream W/input chunks c-outer into 8 PSUM banks.
  - Phase A: sT[k,q] tiles -> exp on ScalarE -> pT (bf16); row sums of P
    accumulate on PE via lhsT=pT slices, rhs=ones (one PSUM bank).
  - Phase B: out2T[vd',q] = (value^T P)^T using natural-layout value tiles
    as lhsT (contraction over k) -- no transposes.
  - Phase C: out[q,:] = (out2T^T @ Wv) * (1/rowsum) + bv, normalization and
    bias fused in one scalar_tensor_tensor on the PSUM->SBUF copy.
  - v3: DMA batching. Per-dma_start issue cost is ~0.6us of engine time plus
    semaphore latency, so v2's ~60 small prologue DMAs gated the first
    matmul to t=33us. v3 packs biases host-side ([128,16] in one DMA),
    loads W in 1-2 multi-chunk 3D-AP DMAs, qT/kT one DMA per 512-col block,
    value 2 k-tiles per DMA, output 2 col-blocks per DMA. bv-broadcast
    matmuls moved after the projections so PE starts on q-proj immediately.

Per-core PE budget @2.4GHz: qproj 65k + kproj 131k + scores 131k +
phaseB 131k + phaseC 65k ~= 523k cycles ~= 218us + ~15us rowsum overhead.
"""
import ml_dtypes
import numpy as np

import concourse.bass as bass
import concourse.bacc as bacc
import concourse.tile as tile
from concourse import mybir
from concourse.bass_utils import run_bass_kernel_spmd

B, S, D = 4, 2048, 1024
KD = 1024
VD = 1024
BETA = 1.0 / float(np.sqrt(D))
N_CORES = 8
QS = S // 2          # per-core query rows (1024)

F32 = mybir.dt.float32
BF16 = mybir.dt.bfloat16

C_D = D // 128       # 8 contraction chunks over D
G_KD = KD // 128     # 8 kd chunks
KT = S // 128        # 16 key tiles
QBLK = 512
NQB = QS // QBLK     # 2 q blocks
NQS = QBLK // 128    # 4 q slices per block
NKB = S // 512       # 4 key col-blocks


def build_kernel():
    nc = bacc.Bacc("TRN2", target_bir_lowering=False, debug=False,
                   num_devices=N_CORES)

    qT = nc.dram_tensor("qT", [D, QS], BF16, kind="ExternalInput").ap()
    kT = nc.dram_tensor("kT", [D, S], BF16, kind="ExternalInput").ap()
    v16 = nc.dram_tensor("v16", [S, VD], BF16, kind="ExternalInput").ap()
    Wq = nc.dram_tensor("Wq", [D, KD], BF16, kind="ExternalInput").ap()
    Wk = nc.dram_tensor("Wk", [D, KD], BF16, kind="ExternalInput").ap()
    Wv16 = nc.dram_tensor("Wv16", [VD, VD], BF16, kind="ExternalInput").ap()
    bqk = nc.dram_tensor("bqk", [128, 16], F32, kind="ExternalInput").ap()
    bv = nc.dram_tensor("bv", [VD], F32, kind="ExternalInput").ap()
    out = nc.dram_tensor("out", [QS, VD], F32, kind="ExternalOutput").ap()

    with tile.TileContext(nc) as tc:
        _body(tc, qT, kT, v16, Wq, Wk, Wv16, bqk, bv, out)
    nc.compile()
    return nc


def _chunked(dram_ap, rows0, nchunk, cols):
    """[nchunk*128, cols] DRAM slice as a [128, nchunk, cols] AP."""
    sl = dram_ap[rows0:rows0 + nchunk * 128, 0:cols] if cols else dram_ap
    return sl.rearrange("(c p) x -> p c x", c=nchunk)


def _body(tc, qT, kT, v16, Wq, Wk, Wv16, bqk, bv, out):
    nc = tc.nc
    Exp = mybir.ActivationFunctionType.Exp
    mult = mybir.AluOpType.mult
    add = mybir.AluOpType.add

    # ---- persistent constants ------------------------------------------
    # constf cols: [0:8]=bqT, [8:16]=bkT, [16:1040]=bvb, [1040:1042]=ones,
    # [1042:1058]=rrec (2 qb x 8), row0 [1058:1186]=ones row,
    # row0 [1186:2210]=bv staging
    const_pool = tc.alloc_tile_pool(name="const", bufs=1)
    constf = const_pool.tile([128, 2210], F32, name="constf")
    bqT = constf[:, 0:8]
    bkT = constf[:, 8:16]
    bvb = constf[:, 16:16 + VD]
    ones_f = constf[:, 1040:1042]
    rrec_all = constf[:, 1042:1058]
    onesrow_f = constf[0:1, 1058:1058 + 128]
    bv_stage = constf[0:1, 1186:1186 + VD]
    onesb = const_pool.tile([128, 2], BF16, name="onesb")

    nc.scalar.dma_start(out=constf[:, 0:16], in_=bqk[:, :])
    nc.scalar.dma_start(out=bv_stage, in_=bv[:])
    nc.vector.memset(ones_f, 1.0)
    nc.vector.memset(onesrow_f, 1.0)
    nc.vector.tensor_copy(onesb[:], ones_f)

    # ---- persistent activations ----------------------------------------
    big_pool = tc.alloc_tile_pool(name="big", bufs=1)
    qTr = big_pool.tile([128, G_KD * QS], BF16, name="qTr")       # 16KB/p
    kTr = big_pool.tile([128, G_KD * S], BF16, name="kTr")        # 32KB/p
    Wv_sb = big_pool.tile([128, C_D * VD], BF16, name="Wv_sb")    # 16KB/p
    pT = big_pool.tile([128, KT * QBLK], BF16, name="pT")         # 16KB/p
    o2T = big_pool.tile([128, C_D * QBLK], BF16, name="o2T")      # 8KB/p
    ost_all = big_pool.tile([128, 2 * 1024], F32, name="ost_all")  # 8KB/p
    ostage = [ost_all[:, i * 1024:(i + 1) * 1024] for i in range(2)]

    # Wv whole in one DMA on gpsimd (needed only at phase C)
    nc.gpsimd.dma_start(out=Wv_sb[:].rearrange("p (c x) -> p c x", c=C_D),
                        in_=_chunked(Wv16, 0, C_D, VD))

    # ---- projection-phase transients -----------------------------------
    proj_pool = tc.alloc_tile_pool(name="proj", bufs=1)
    Wq_sb = proj_pool.tile([128, C_D * KD], BF16, name="Wq_sb")   # 16KB/p
    Wk_sb = proj_pool.tile([128, C_D * KD], BF16, name="Wk_sb")   # 16KB/p
    qblk = [proj_pool.tile([128, C_D * 512], BF16, name=f"qb{i}")
            for i in range(NQB)]                                  # 2x8KB/p
    kblk = [proj_pool.tile([128, C_D * 512], BF16, name=f"kb{i}")
            for i in range(NKB)]                                  # 4x8KB/p

    # sync queue: Wq halves interleaved with qT blocks, then kT blocks 2-3
    Wq_v = Wq_sb[:].rearrange("p (c x) -> p c x", c=C_D)
    Wk_v = Wk_sb[:].rearrange("p (c x) -> p c x", c=C_D)
    for h in range(2):
        nc.sync.dma_start(out=Wq_v[:, 4 * h:4 * h + 4, :],
                          in_=_chunked(Wq, h * 512, 4, KD))
        nc.sync.dma_start(
            out=qblk[h][:].rearrange("p (c x) -> p c x", c=C_D),
            in_=qT[:, h * 512:(h + 1) * 512].rearrange(
                "(c p) x -> p c x", c=C_D))
    # scalar queue: biases above, then Wk whole, kT blocks 0-1
    nc.scalar.dma_start(out=Wk_v[:, :, :], in_=_chunked(Wk, 0, C_D, KD))
    for blk in range(NKB):
        eng = nc.scalar if blk < 2 else nc.sync
        eng.dma_start(
            out=kblk[blk][:].rearrange("p (c x) -> p c x", c=C_D),
            in_=kT[:, blk * 512:(blk + 1) * 512].rearrange(
                "(c p) x -> p c x", c=C_D))

    psPro = tc.alloc_tile_pool(name="psPro", bufs=1, space="PSUM")

    # ---- q projection: qTr[kd, q] = (Wq^T qT) + bq ----------------------
    for blk in range(NQB):
        pps = [psPro.tile([128, 512], F32, name=f"qp{blk}_{g}", tag="pp",
                          bufs=8) for g in range(G_KD)]
        for c in range(C_D):
            for g in range(G_KD):
                nc.tensor.matmul(
                    pps[g][:],
                    Wq_sb[:, c * KD + g * 128:c * KD + (g + 1) * 128],
                    qblk[blk][:, c * 512:(c + 1) * 512],
                    start=(c == 0), stop=(c == C_D - 1))
        for g in range(G_KD):
            nc.vector.tensor_scalar(
                out=qTr[:, g * QS + blk * 512:g * QS + (blk + 1) * 512],
                in0=pps[g][:], scalar1=bqT[:, g:g + 1], scalar2=None, op0=add)

    # ---- k projection: kTr[kd, k] = (Wk^T kT) + bk ----------------------
    for blk in range(NKB):
        pps = [psPro.tile([128, 512], F32, name=f"kp{blk}_{g}", tag="pp",
                          bufs=8) for g in range(G_KD)]
        for c in range(C_D):
            for g in range(G_KD):
                nc.tensor.matmul(
                    pps[g][:],
                    Wk_sb[:, c * KD + g * 128:c * KD + (g + 1) * 128],
                    kblk[blk][:, c * 512:(c + 1) * 512],
                    start=(c == 0), stop=(c == C_D - 1))
        for g in range(G_KD):
            nc.vector.tensor_scalar(
                out=kTr[:, g * S + blk * 512:g * S + (blk + 1) * 512],
                in0=pps[g][:], scalar1=bkT[:, g:g + 1], scalar2=None, op0=add)

    # bv broadcast to all partitions via K=1 fp32 matmul (off critical path)
    for n in range(VD // 512):
        bc_ps = psPro.tile([128, 512], F32, name="bc_ps", tag="pp", bufs=8)
        nc.tensor.matmul(bc_ps[:], onesrow_f,
                         bv_stage[:, n * 512:(n + 1) * 512],
                         start=True, stop=True)
        nc.vector.tensor_copy(bvb[:, n * 512:(n + 1) * 512], bc_ps[:])

    psPro.release()
    proj_pool.release()

    # ===== main attention loop ==========================================
    # PSUM: sT(2) + rs(1) + acc(4) = 7 banks.
    psM = tc.alloc_tile_pool(name="psM", bufs=1, space="PSUM")
    rs_ps = psM.tile([128, 2 * NQS], F32, name="rs_ps", tag="rs")

    for qb in range(NQB):
        q0 = qb * QBLK
        # ---- phase A: sT = kTr^T qTr -> exp -> pT ; rowsums on PE ------
        for kt in range(KT):
            sT = psM.tile([128, QBLK], F32, name=f"sT{qb}_{kt}", tag="sT",
                          bufs=2)
            for g in range(G_KD):
                nc.tensor.matmul(
                    sT[:],
                    kTr[:, g * S + kt * 128:g * S + (kt + 1) * 128],
                    qTr[:, g * QS + q0:g * QS + q0 + QBLK],
                    start=(g == 0), stop=(g == G_KD - 1))
            nc.scalar.activation(pT[:, kt * QBLK:(kt + 1) * QBLK], sT[:],
                                 Exp, scale=float(BETA))
            for qs in range(NQS):
                nc.tensor.matmul(
                    rs_ps[:, 2 * qs:2 * qs + 2],
                    pT[:, kt * QBLK + qs * 128:kt * QBLK + (qs + 1) * 128],
                    onesb[:],
                    start=(kt == 0 and qs == 0),
                    stop=(kt == KT - 1 and qs == NQS - 1),
                    skip_group_check=True)
        rrec = rrec_all[:, qb * 2 * NQS:(qb + 1) * 2 * NQS]
        nc.vector.reciprocal(rrec, rs_ps[:])

        # ---- phase B: o2T[vd', q] = (value^T P)^T via lhsT=value tiles --
        for p in range(2):
            accs = [psM.tile([128, QBLK], F32, name=f"o2{qb}_{p}_{u}",
                             tag="acc", bufs=4) for u in range(4)]
            for kt2 in range(KT // 2):
                vt = big_pool.tile([128, 2 * 512], BF16,
                                   name=f"vt{qb}_{p}_{kt2}", tag="vring",
                                   bufs=4)
                nc.gpsimd.dma_start(
                    out=vt[:].rearrange("p (c x) -> p c x", c=2),
                    in_=v16[kt2 * 256:(kt2 + 1) * 256,
                            p * 512:(p + 1) * 512].rearrange(
                        "(c p) x -> p c x", c=2))
                for j in range(2):
                    kt = 2 * kt2 + j
                    for u in range(4):
                        nc.tensor.matmul(
                            accs[u][:],
                            vt[:, j * 512 + u * 128:j * 512 + (u + 1) * 128],
                            pT[:, kt * QBLK:(kt + 1) * QBLK],
                            start=(kt == 0), stop=(kt == KT - 1))
            for u in range(4):
                nc.vector.tensor_copy(
                    o2T[:, (4 * p + u) * QBLK:(4 * p + u + 1) * QBLK],
                    accs[u][:])

        # ---- phase C: out = (o2T^T Wv) * rrec + bv ----------------------
        for qs in range(NQS):
            ost = ostage[qs % 2]
            for col in range(2):
                op = psM.tile([128, 512], F32, name=f"op{qb}_{qs}_{col}",
                              tag="acc", bufs=4)
                for cp in range(C_D):
                    nc.tensor.matmul(
                        op[:],
                        o2T[:, cp * QBLK + qs * 128:cp * QBLK + (qs + 1) * 128],
                        Wv_sb[:, cp * VD + col * 512:cp * VD + (col + 1) * 512],
                        start=(cp == 0), stop=(cp == C_D - 1))
                nc.vector.scalar_tensor_tensor(
                    out=ost[:, col * 512:(col + 1) * 512], in0=op[:],
                    scalar=rrec[:, 2 * qs:2 * qs + 1],
                    in1=bvb[:, col * 512:(col + 1) * 512], op0=mult, op1=add)
            nc.sync.dma_start(
                out=out[q0 + qs * 128:q0 + (qs + 1) * 128, :], in_=ost[:])

    psM.release()
    big_pool.release()
    const_pool.release()


_NC_CACHE = {}


def _get_nc():
    if "nc" not in _NC_CACHE:
        _NC_CACHE["nc"] = build_kernel()
    return _NC_CACHE["nc"]


def kernel(query, key, value, Wq, bq, Wk, bk, Wv, bv):
    query = np.asarray(query, dtype=np.float32)
    key = np.asarray(key, dtype=np.float32)
    value = np.asarray(value, dtype=np.float32)
    Wq = np.asarray(Wq, dtype=np.float32)
    Wk = np.asarray(Wk, dtype=np.float32)
    Wv = np.asarray(Wv, dtype=np.float32)
    bq = np.asarray(bq, dtype=np.float32)
    bk = np.asarray(bk, dtype=np.float32)
    bv = np.ascontiguousarray(np.asarray(bv, dtype=np.float32))

    nc = _get_nc()
    in_maps = make_in_maps(query, key, value, Wq, bq, Wk, bk, Wv, bv)
    res = run_bass_kernel_spmd(nc, in_maps, list(range(N_CORES)))
    outp = np.empty((B, S, VD), dtype=np.float32)
    for core in range(N_CORES):
        b, h = divmod(core, 2)
        outp[b, h * QS:(h + 1) * QS, :] = res.results[core]["out"]
    return outp


def make_in_maps(query, key, value, Wq, bq, Wk, bk, Wv, bv):
    bf16 = ml_dtypes.bfloat16
    Wq16 = Wq.astype(bf16)
    Wk16 = Wk.astype(bf16)
    Wv16 = Wv.astype(bf16)
    bqk = np.ascontiguousarray(
        np.concatenate([bq.reshape(8, 128).T, bk.reshape(8, 128).T], axis=1)
        .astype(np.float32))
    kTs = [np.ascontiguousarray(key[b].T.astype(bf16)) for b in range(B)]
    v16s = [np.ascontiguousarray(value[b].astype(bf16)) for b in range(B)]
    in_maps = []
    for core in range(N_CORES):
        b, h = divmod(core, 2)
        in_maps.append({
            "qT": np.ascontiguousarray(query[b, h * QS:(h + 1) * QS, :].T
                                       .astype(bf16)),
            "kT": kTs[b],
            "v16": v16s[b],
            "Wq": Wq16, "Wk": Wk16, "Wv16": Wv16,
            "bqk": bqk, "bv": bv,
        })
    return in_maps
